# revision 27
# baseline (speedup 1.0000x reference)
"""BinaryTreeLSTM on 8 Trainium2 NeuronCores.

Data-parallel over the leaf batch: core d owns leaves [1024d, 1024d+1024)
in BIT-REVERSED order and folds its subtree feature-major through 4 merge
levels (1024 -> 64 nodes); the 8x64 per-core subtree roots are combined on
the host for the remaining 9 (tiny, serial) levels.

Bit-reversal makes every level's left children land at free columns [0:B]
and right children at [B:2B], so all levels use identical feature-major
compute: state is [128 partitions = m-features, 2 chunks, nodes], weights
are the stationary matmul operand (bf16 -> fast weight load), h streams as
the moving operand (f32r, single-pass PE), and child reads are contiguous
slices. No transposes, no SBUF-to-SBUF gathers, no node-major regime.

Bias handling: bx is folded into the leaf matmul via an augmented ones-row
in the embedding chunk / bx-row in the Wx chunk; the internal-node pad
projection px is host-precomputed and applied via the ACT per-partition
bias (wide levels) or a rank-1 PE pass (narrow levels, prefetchable).
"""

import numpy as np

IN_DIM = 300
MEM_DIM = 256
N_LEAVES = 8192
N_CORES = 8
LPC = N_LEAVES // N_CORES  # 1024 leaves per core
B_STOP = 128               # per-core nodes returned to the host
GL = 256                   # leaf/level node-chunk size

# 5-gate order [u, i, lf, rf, o]; lf and rf share the fx slice of px
_PX5SRC = [0, 256, 512, 512, 768]

_CACHE = {}


def _bitrev_perm(n):
    bits = n.bit_length() - 1
    p = np.arange(n)
    r = np.zeros(n, dtype=np.int64)
    for b in range(bits):
        r |= ((p >> b) & 1) << (bits - 1 - b)
    return r


def _build():
    import concourse.bacc as bacc
    import concourse.mybir as mybir
    import concourse.tile as tile

    f32 = mybir.dt.float32
    f32r = mybir.dt.float32r
    f16 = mybir.dt.float16
    AF = mybir.ActivationFunctionType

    nc = bacc.Bacc("TRN2", target_bir_lowering=False, debug=False,
                   num_devices=N_CORES)

    # k-chunked inputs (separate tensors => DMA/dependency granularity)
    embsT = [nc.dram_tensor(f"embsT{k}", [128, LPC], f16,
                            kind="ExternalInput").ap() for k in range(3)]
    WxT = [nc.dram_tensor(f"WxT{k}", [128, 1024], f16,
                          kind="ExternalInput").ap() for k in range(3)]
    WlT = nc.dram_tensor("WlT", [128, 2 * 1280], f16, kind="ExternalInput").ap()
    WrT = nc.dram_tensor("WrT", [128, 2 * 1280], f16, kind="ExternalInput").ap()
    px5fm = nc.dram_tensor("px5fm", [128, 10], f32, kind="ExternalInput").ap()
    px5r = nc.dram_tensor("px5r", [1, 1280], f16, kind="ExternalInput").ap()
    out = nc.dram_tensor("out", [256, 2 * B_STOP], f32, kind="ExternalOutput").ap()

    with tile.TileContext(nc) as tc:
        with (
            tc.tile_pool(name="const", bufs=1) as const,
            tc.tile_pool(name="state", bufs=1) as state,
            tc.tile_pool(name="gates", bufs=2) as gates,
            tc.tile_pool(name="psum", bufs=1, space="PSUM") as psum,
        ):
            v2 = lambda t: t.rearrange("p (c n) -> p c n", c=2)

            # HAM warm-up source: memset, no DMA dependency, scheduled at
            # the very front so dummy matmuls can warm the PE clock gate
            # while input DMAs stream
            warm_sb = const.tile([128, 1024], f16, tag="warm")
            warm_ps = psum.tile([128, 512], f32, tag="u", bufs=1, name="warm")
            with tc.high_priority():
                nc.vector.memset(warm_sb[:, :], 1.0)
                for wi in range(7):
                    nc.tensor.matmul(warm_ps[:, :], warm_sb[:, 0:128],
                                     warm_sb[:, 0:512],
                                     start=(wi == 0), stop=(wi == 6))

            # ---- input DMAs: leaf tensors first, spread across queues ----
            WxT_sb = [const.tile([128, 1024], f16, name=f"wx{k}",
                             tag=f"wx{k}") for k in range(3)]
            embsT_sb = [const.tile([128, LPC], f16, name=f"em{k}",
                        tag=f"em{k}") for k in range(3)]
            # first-needed-first per queue; embsT ships as 512-col halves
            # so the first leaf chunk's inputs land early
            nc.scalar.dma_start(WxT_sb[0][:, :], WxT[0][:, :])
            nc.sync.dma_start(WxT_sb[1][:, :], WxT[1][:, :])
            for k in range(3):
                nc.gpsimd.dma_start(embsT_sb[k][:, 0:512], embsT[k][:, 0:512])
            nc.scalar.dma_start(WxT_sb[2][:, :], WxT[2][:, :])
            for k in range(3):
                nc.sync.dma_start(embsT_sb[k][:, 512:1024],
                                  embsT[k][:, 512:1024])
            WlT_sb = const.tile([128, 2 * 1280], f16, tag="wl")
            WrT_sb = const.tile([128, 2 * 1280], f16, tag="wr")
            px5fm_sb = const.tile([128, 10], f32, tag="pxf")
            px5r_sb = const.tile([1, 1280], f16, tag="pxr")
            nc.scalar.dma_start(WlT_sb[:, :], WlT[:, :])
            nc.gpsimd.dma_start(WrT_sb[:, :], WrT[:, :])
            nc.sync.dma_start(px5fm_sb[:, :], px5fm[:, :])
            nc.sync.dma_start(px5r_sb[:, :], px5r[:, :])

            ones_sb = warm_sb  # all-ones f16, used by the rank-1 px pass
            GATE_FNS = [AF.Tanh, AF.Sigmoid, AF.Sigmoid, AF.Sigmoid, AF.Sigmoid]
            GTAG = ["u", "i", "lf", "rf", "o"]

            # ---- leaf phase: 1024 leaves -> c0, h0 ----
            # ki-outer so matmuls start as soon as chunk-0 DMAs land; each
            # gate's PSUM tile holds both halves (one bank) in a single
            # accumulation group (start only on the very first matmul)
            c0 = state.tile([128, 2 * LPC], f16, name="c_leaf", tag="c_leaf")
            h0 = state.tile([128, 2 * LPC], f16, name="h_leaf", tag="h_leaf")
            c0_3, h0_3 = v2(c0), v2(h0)
            KR = [128, 128, 45]  # rows per k-chunk (chunk 2: 44 data + bias)
            GLF = 512
            LEAF_G = (("u", 0, AF.Tanh), ("i", 1, AF.Sigmoid),
                      ("o", 3, AF.Sigmoid))
            with nc.named_scope("leaf"):
                for sg in range(LPC // GLF):
                    ps = {}
                    for gname, gm, fn in LEAF_G:
                        ps[gname] = psum.tile([128, 2, GLF], f32, tag=gname,
                                              name=f"ps_{gname}{sg}", bufs=1)
                    for ki in range(3):
                        for gname, gm, fn in LEAF_G:
                            for half in range(2):
                                m = gm * 2 + half
                                nc.tensor.matmul(
                                    ps[gname][:, half, :],
                                    WxT_sb[ki][0:KR[ki], m * 128:(m + 1) * 128],
                                    embsT_sb[ki][0:KR[ki],
                                                 sg * GLF:(sg + 1) * GLF],
                                    start=(ki == 0), stop=(ki == 2))
                    sb = {}
                    for gname, gm, fn in LEAF_G:
                        t = gates.tile([128, 2 * GLF], f16, tag=gname,
                                       name=f"g_{gname}{sg}")
                        nc.scalar.activation(v2(t), ps[gname][:, :, :], fn)
                        sb[gname] = t
                    tht = gates.tile([128, 2 * GLF], f16, tag="th", name=f"th{sg}")
                    cs = c0_3[:, :, sg * GLF:(sg + 1) * GLF]
                    nc.vector.tensor_mul(cs, v2(sb["i"]), v2(sb["u"]))
                    nc.scalar.activation(v2(tht), cs, AF.Tanh)
                    nc.vector.tensor_mul(h0_3[:, :, sg * GLF:(sg + 1) * GLF],
                                         v2(sb["o"]), v2(tht))

            # ---- merge levels, all feature-major ----
            def fm_level(h_prev, c_prev, B, lvl):
                last = (B == B_STOP)
                h_n = state.tile([128, 2 * B], f32 if last else f16,
                                 name=f"h{lvl}", tag=f"h{lvl}")
                c_n = state.tile([128, 2 * B], f32 if last else f16,
                                 name=f"c{lvl}", tag=f"c{lvl}")
                hp3, cp3 = v2(h_prev), v2(c_prev)
                use_bias = B >= 512
                CH = 256 if B > 256 else B
                for g0 in range(0, B, CH):
                    G = min(CH, B - g0)
                    sfx = f"{lvl}_{g0}"
                    sb = {}
                    for gi in range(5):
                        g = gates.tile([128, 2 * G], f16, tag=GTAG[gi],
                                       name=f"g_{GTAG[gi]}{sfx}")
                        fused = gi in (0, 1, 4)
                        if fused:
                            t = psum.tile([128, 2, G], f32, tag=GTAG[gi],
                                          name=f"ps{GTAG[gi]}{sfx}", bufs=1,
                                          padded_shape=[128, 2, 512])
                        for half in range(2):
                            m = gi * 2 + half
                            if fused:
                                dst = t[:, half, :]
                            else:
                                th_ = psum.tile([128, G], f32, tag=GTAG[gi],
                                                name=f"ps{GTAG[gi]}{sfx}_{half}",
                                                bufs=1)
                                dst = th_[:, :]
                            if not use_bias:
                                nc.tensor.matmul(
                                    dst, px5r_sb[0:1, m * 128:(m + 1) * 128],
                                    ones_sb[0:1, 0:G],
                                    start=True, stop=False)
                            for ki in range(4):
                                side, kc = ki // 2, ki % 2
                                W = WlT_sb if side == 0 else WrT_sb
                                nc.tensor.matmul(
                                    dst,
                                    W[:, kc * 1280 + m * 128:
                                      kc * 1280 + (m + 1) * 128],
                                    hp3[:, kc, side * B + g0:side * B + g0 + G],
                                    start=(use_bias and ki == 0),
                                    stop=(ki == 3))
                            if use_bias:
                                nc.scalar.activation(
                                    g[:, half * G:(half + 1) * G], dst,
                                    GATE_FNS[gi],
                                    bias=px5fm_sb[:, gi * 2 + half:
                                                  gi * 2 + half + 1])
                            elif not fused:
                                nc.scalar.activation(
                                    g[:, half * G:(half + 1) * G], dst,
                                    GATE_FNS[gi])
                        if fused and not use_bias:
                            nc.scalar.activation(v2(g), t[:, :, :],
                                                 GATE_FNS[gi])
                        sb[gi] = g
                    x1 = gates.tile([128, 2 * G], f16, tag="x1", name=f"x1{sfx}")
                    x2 = gates.tile([128, 2 * G], f16, tag="x2", name=f"x2{sfx}")
                    x3 = gates.tile([128, 2 * G], f16, tag="x3", name=f"x3{sfx}")
                    s1 = gates.tile([128, 2 * G], f16, tag="s1", name=f"s1{sfx}")
                    tht = gates.tile([128, 2 * G], f16, tag="th",
                                     name=f"th{sfx}")
                    lc = cp3[:, :, g0:g0 + G]
                    rc = cp3[:, :, B + g0:B + g0 + G]
                    nc.vector.tensor_mul(v2(x1), v2(sb[1]), v2(sb[0]))
                    nc.vector.tensor_mul(v2(x2), v2(sb[2]), lc)
                    nc.vector.tensor_mul(v2(x3), v2(sb[3]), rc)
                    nc.vector.tensor_add(v2(s1), v2(x1), v2(x2))
                    cs = v2(c_n)[:, :, g0:g0 + G]
                    nc.vector.tensor_add(cs, v2(s1), v2(x3))
                    nc.scalar.activation(v2(tht), cs, AF.Tanh)
                    nc.vector.tensor_mul(v2(h_n)[:, :, g0:g0 + G],
                                         v2(sb[4]), v2(tht))
                return h_n, c_n

            h, c = h0, c0
            B = LPC
            lvl = 0
            while B > B_STOP:
                B //= 2
                with nc.named_scope(f"L{lvl}_B{B}"):
                    h, c = fm_level(h, c, B, lvl)
                lvl += 1

            nc.sync.dma_start(out[0:128, :], c[:, :])
            nc.scalar.dma_start(out[128:256, :], h[:, :])

    nc.compile()
    return nc


def _get_nc():
    if "nc" not in _CACHE:
        _CACHE["nc"] = _build()
    return _CACHE["nc"]


def kernel(embs, Wx, bx, Wl, Wr, emb_table, _trace=False, _trace_kwargs=None):
    from concourse.bass_utils import run_bass_kernel_spmd

    embs = np.asarray(embs, dtype=np.float32)
    Wx = np.asarray(Wx, dtype=np.float32)
    bx = np.asarray(bx, dtype=np.float32)
    Wl = np.asarray(Wl, dtype=np.float32)
    Wr = np.asarray(Wr, dtype=np.float32)
    emb_table = np.asarray(emb_table, dtype=np.float32)

    WxT = np.ascontiguousarray(Wx.T)                      # [300, 1024]
    WlT = np.ascontiguousarray(Wl.T)                      # [256, 1280]
    WrT = np.ascontiguousarray(Wr.T)

    # Wx chunks with bx folded in as an extra contraction row (row 44 of
    # chunk 2, matching the ones-row in the embedding chunk)
    WxT_ch = []
    for k in range(2):
        WxT_ch.append(np.ascontiguousarray(
            WxT[128 * k:128 * (k + 1)].astype(np.float16)))
    w2 = np.zeros((128, 1024), dtype=np.float16)
    w2[0:44] = WxT[256:300].astype(np.float16)
    w2[44] = bx.astype(np.float16)
    WxT_ch.append(w2)

    # weight images [128, 2*1280] (k-chunks side by side), fp16
    WlT_img = np.ascontiguousarray(
        np.concatenate([WlT[0:128], WlT[128:256]], axis=1).astype(np.float16))
    WrT_img = np.ascontiguousarray(
        np.concatenate([WrT[0:128], WrT[128:256]], axis=1).astype(np.float16))

    # pad-node x-projection, expanded to the 5-gate layout
    px = emb_table[-1] @ WxT + bx                          # [1024]
    px5 = np.concatenate([px[s:s + 256] for s in _PX5SRC]) # [1280]
    px5r = np.ascontiguousarray(px5.reshape(1, 1280).astype(np.float16))
    px5fm = np.ascontiguousarray(px5.reshape(10, 128).T)   # [128, 10]

    perm = _bitrev_perm(LPC)
    in_maps = []
    for d in range(N_CORES):
        shard = embs[d * LPC:(d + 1) * LPC][perm].T.astype(np.float16)
        e2 = np.zeros((128, LPC), dtype=np.float16)
        e2[0:44] = shard[256:300]
        e2[44] = 1.0
        in_maps.append({
            "embsT0": np.ascontiguousarray(shard[0:128]),
            "embsT1": np.ascontiguousarray(shard[128:256]),
            "embsT2": e2,
            "WxT0": WxT_ch[0], "WxT1": WxT_ch[1], "WxT2": WxT_ch[2],
            "WlT": WlT_img, "WrT": WrT_img,
            "px5fm": px5fm, "px5r": px5r,
        })

    nc = _get_nc()
    res = run_bass_kernel_spmd(nc, in_maps, list(range(N_CORES)),
                               trace=_trace, **(_trace_kwargs or {}))
    _CACHE["last_result"] = res

    # ---- unshard: un-bit-reverse, then fold the remaining levels ----
    rperm = _bitrev_perm(B_STOP)  # position p holds node rperm[p]
    cs, hs = [], []
    for d in range(N_CORES):
        o = np.asarray(res.results[d]["out"], dtype=np.float32)
        cf = o[0:128].reshape(128, 2, B_STOP)
        hf = o[128:256].reshape(128, 2, B_STOP)
        c_nm = np.concatenate([cf[:, 0, :], cf[:, 1, :]], axis=0).T  # [B,256]
        h_nm = np.concatenate([hf[:, 0, :], hf[:, 1, :]], axis=0).T
        inv = np.empty(B_STOP, dtype=np.int64)
        inv[rperm] = np.arange(B_STOP)
        cs.append(c_nm[inv])   # node order
        hs.append(h_nm[inv])
    c = np.concatenate(cs, axis=0)  # [512, 256]
    h = np.concatenate(hs, axis=0)
    m = MEM_DIM

    def sig(x):
        return 1.0 / (1.0 + np.exp(-x))

    while c.shape[0] > 1:
        lg = h[0::2] @ WlT
        rg = h[1::2] @ WrT
        u = np.tanh(px[0:m] + lg[:, 0:m] + rg[:, 0:m])
        i = sig(px[m:2 * m] + lg[:, m:2 * m] + rg[:, m:2 * m])
        lf = sig(px[2 * m:3 * m] + lg[:, 2 * m:3 * m] + rg[:, 2 * m:3 * m])
        rf = sig(px[2 * m:3 * m] + lg[:, 3 * m:4 * m] + rg[:, 3 * m:4 * m])
        o = sig(px[3 * m:4 * m] + lg[:, 4 * m:5 * m] + rg[:, 4 * m:5 * m])
        c = i * u + lf * c[0::2] + rf * c[1::2]
        h = o * np.tanh(c)
    return np.stack([c, h]).astype(np.float32)


# revision 28
# speedup vs baseline: 1.0099x; 1.0099x over previous
"""BinaryTreeLSTM on 8 Trainium2 NeuronCores.

Data-parallel over the leaf batch: core d owns leaves [1024d, 1024d+1024)
in BIT-REVERSED order and folds its subtree feature-major through 4 merge
levels (1024 -> 64 nodes); the 8x64 per-core subtree roots are combined on
the host for the remaining 9 (tiny, serial) levels.

Bit-reversal makes every level's left children land at free columns [0:B]
and right children at [B:2B], so all levels use identical feature-major
compute: state is [128 partitions = m-features, 2 chunks, nodes], weights
are the stationary matmul operand (bf16 -> fast weight load), h streams as
the moving operand (f32r, single-pass PE), and child reads are contiguous
slices. No transposes, no SBUF-to-SBUF gathers, no node-major regime.

Bias handling: bx is folded into the leaf matmul via an augmented ones-row
in the embedding chunk / bx-row in the Wx chunk; the internal-node pad
projection px is host-precomputed and applied via the ACT per-partition
bias (wide levels) or a rank-1 PE pass (narrow levels, prefetchable).
"""

import numpy as np

IN_DIM = 300
MEM_DIM = 256
N_LEAVES = 8192
N_CORES = 8
LPC = N_LEAVES // N_CORES  # 1024 leaves per core
B_STOP = 128               # per-core nodes returned to the host
GL = 256                   # leaf/level node-chunk size

# 5-gate order [u, i, lf, rf, o]; lf and rf share the fx slice of px
_PX5SRC = [0, 256, 512, 512, 768]

_CACHE = {}


def _bitrev_perm(n):
    bits = n.bit_length() - 1
    p = np.arange(n)
    r = np.zeros(n, dtype=np.int64)
    for b in range(bits):
        r |= ((p >> b) & 1) << (bits - 1 - b)
    return r


def _build():
    import concourse.bacc as bacc
    import concourse.mybir as mybir
    import concourse.tile as tile

    f32 = mybir.dt.float32
    f32r = mybir.dt.float32r
    f16 = mybir.dt.float16
    AF = mybir.ActivationFunctionType

    nc = bacc.Bacc("TRN2", target_bir_lowering=False, debug=False,
                   num_devices=N_CORES)

    # k-chunked inputs (separate tensors => DMA/dependency granularity)
    embsT = [nc.dram_tensor(f"embsT{k}", [128, LPC], f16,
                            kind="ExternalInput").ap() for k in range(3)]
    WxT = [nc.dram_tensor(f"WxT{k}", [128, 1024], f16,
                          kind="ExternalInput").ap() for k in range(3)]
    WlT = nc.dram_tensor("WlT", [128, 2 * 1280], f16, kind="ExternalInput").ap()
    WrT = nc.dram_tensor("WrT", [128, 2 * 1280], f16, kind="ExternalInput").ap()
    px5fm = nc.dram_tensor("px5fm", [128, 10], f32, kind="ExternalInput").ap()
    px5r = nc.dram_tensor("px5r", [1, 1280], f16, kind="ExternalInput").ap()
    out = nc.dram_tensor("out", [256, 2 * B_STOP], f32, kind="ExternalOutput").ap()

    with tile.TileContext(nc) as tc:
        with (
            tc.tile_pool(name="const", bufs=1) as const,
            tc.tile_pool(name="state", bufs=1) as state,
            tc.tile_pool(name="gates", bufs=2) as gates,
            tc.tile_pool(name="psum", bufs=1, space="PSUM") as psum,
        ):
            v2 = lambda t: t.rearrange("p (c n) -> p c n", c=2)

            # HAM warm-up source: memset, no DMA dependency, scheduled at
            # the very front so dummy matmuls can warm the PE clock gate
            # while input DMAs stream
            warm_sb = const.tile([128, 1024], f16, tag="warm")
            warm_ps = psum.tile([128, 512], f32, tag="u", bufs=2, name="warm")
            with tc.high_priority():
                nc.vector.memset(warm_sb[:, :], 1.0)
                for wi in range(7):
                    nc.tensor.matmul(warm_ps[:, :], warm_sb[:, 0:128],
                                     warm_sb[:, 0:512],
                                     start=(wi == 0), stop=(wi == 6))

            # ---- input DMAs: leaf tensors first, spread across queues ----
            WxT_sb = [const.tile([128, 1024], f16, name=f"wx{k}",
                             tag=f"wx{k}") for k in range(3)]
            embsT_sb = [const.tile([128, LPC], f16, name=f"em{k}",
                        tag=f"em{k}") for k in range(3)]
            # first-needed-first per queue; embsT ships as 512-col halves
            # so the first leaf chunk's inputs land early
            nc.scalar.dma_start(WxT_sb[0][:, :], WxT[0][:, :])
            nc.sync.dma_start(WxT_sb[1][:, :], WxT[1][:, :])
            for k in range(3):
                nc.gpsimd.dma_start(embsT_sb[k][:, 0:512], embsT[k][:, 0:512])
            nc.scalar.dma_start(WxT_sb[2][:, :], WxT[2][:, :])
            for k in range(3):
                nc.sync.dma_start(embsT_sb[k][:, 512:1024],
                                  embsT[k][:, 512:1024])
            WlT_sb = const.tile([128, 2 * 1280], f16, tag="wl")
            WrT_sb = const.tile([128, 2 * 1280], f16, tag="wr")
            px5fm_sb = const.tile([128, 10], f32, tag="pxf")
            px5r_sb = const.tile([1, 1280], f16, tag="pxr")
            nc.scalar.dma_start(WlT_sb[:, :], WlT[:, :])
            nc.gpsimd.dma_start(WrT_sb[:, :], WrT[:, :])
            nc.sync.dma_start(px5fm_sb[:, :], px5fm[:, :])
            nc.sync.dma_start(px5r_sb[:, :], px5r[:, :])

            ones_sb = warm_sb  # all-ones f16, used by the rank-1 px pass
            GATE_FNS = [AF.Tanh, AF.Sigmoid, AF.Sigmoid, AF.Sigmoid, AF.Sigmoid]
            GTAG = ["u", "i", "lf", "rf", "o"]

            # ---- leaf phase: 1024 leaves -> c0, h0 ----
            # ki-outer so matmuls start as soon as chunk-0 DMAs land; each
            # gate's PSUM tile holds both halves (one bank) in a single
            # accumulation group (start only on the very first matmul)
            c0 = state.tile([128, 2 * LPC], f16, name="c_leaf", tag="c_leaf")
            h0 = state.tile([128, 2 * LPC], f16, name="h_leaf", tag="h_leaf")
            c0_3, h0_3 = v2(c0), v2(h0)
            KR = [128, 128, 45]  # rows per k-chunk (chunk 2: 44 data + bias)
            GLF = 512
            LEAF_G = (("u", 0, AF.Tanh), ("i", 1, AF.Sigmoid),
                      ("o", 3, AF.Sigmoid))
            with nc.named_scope("leaf"):
                for sg in range(LPC // GLF):
                    ps = {}
                    for gname, gm, fn in LEAF_G:
                        for half in range(2):
                            ps[gname, half] = psum.tile(
                                [128, GLF], f32, tag=gname,
                                name=f"ps_{gname}{sg}_{half}", bufs=2)
                    for ki in range(3):
                        for gname, gm, fn in LEAF_G:
                            for half in range(2):
                                m = gm * 2 + half
                                nc.tensor.matmul(
                                    ps[gname, half][:, :],
                                    WxT_sb[ki][0:KR[ki], m * 128:(m + 1) * 128],
                                    embsT_sb[ki][0:KR[ki],
                                                 sg * GLF:(sg + 1) * GLF],
                                    start=(ki == 0), stop=(ki == 2))
                    sb = {}
                    for gname, gm, fn in LEAF_G:
                        t = gates.tile([128, 2 * GLF], f16, tag=gname,
                                       name=f"g_{gname}{sg}")
                        for half in range(2):
                            nc.scalar.activation(
                                t[:, half * GLF:(half + 1) * GLF],
                                ps[gname, half][:, :], fn)
                        sb[gname] = t
                    tht = gates.tile([128, 2 * GLF], f16, tag="th", name=f"th{sg}")
                    cs = c0_3[:, :, sg * GLF:(sg + 1) * GLF]
                    nc.vector.tensor_mul(cs, v2(sb["i"]), v2(sb["u"]))
                    nc.scalar.activation(v2(tht), cs, AF.Tanh)
                    nc.vector.tensor_mul(h0_3[:, :, sg * GLF:(sg + 1) * GLF],
                                         v2(sb["o"]), v2(tht))

            # ---- merge levels, all feature-major ----
            def fm_level(h_prev, c_prev, B, lvl):
                last = (B == B_STOP)
                h_n = state.tile([128, 2 * B], f32 if last else f16,
                                 name=f"h{lvl}", tag=f"h{lvl}")
                c_n = state.tile([128, 2 * B], f32 if last else f16,
                                 name=f"c{lvl}", tag=f"c{lvl}")
                hp3, cp3 = v2(h_prev), v2(c_prev)
                use_bias = B >= 256
                CH = 256 if B > 256 else (128 if B > 128 else B)
                for g0 in range(0, B, CH):
                    G = min(CH, B - g0)
                    sfx = f"{lvl}_{g0}"
                    sb = {}
                    for gi in range(5):
                        g = gates.tile([128, 2 * G], f16, tag=GTAG[gi],
                                       name=f"g_{GTAG[gi]}{sfx}")
                        for half in range(2):
                            m = gi * 2 + half
                            t = psum.tile([128, G], f32, tag=GTAG[gi],
                                          name=f"ps{GTAG[gi]}{sfx}_{half}",
                                          bufs=2 if gi in (0, 1, 4) else 1)
                            if not use_bias:
                                nc.tensor.matmul(
                                    t[:, :],
                                    px5r_sb[0:1, m * 128:(m + 1) * 128],
                                    ones_sb[0:1, 0:G],
                                    start=True, stop=False)
                            for ki in range(4):
                                side, kc = ki // 2, ki % 2
                                W = WlT_sb if side == 0 else WrT_sb
                                nc.tensor.matmul(
                                    t[:, :],
                                    W[:, kc * 1280 + m * 128:
                                      kc * 1280 + (m + 1) * 128],
                                    hp3[:, kc, side * B + g0:side * B + g0 + G],
                                    start=(use_bias and ki == 0),
                                    stop=(ki == 3))
                            if use_bias:
                                nc.scalar.activation(
                                    g[:, half * G:(half + 1) * G], t[:, :],
                                    GATE_FNS[gi],
                                    bias=px5fm_sb[:, gi * 2 + half:
                                                  gi * 2 + half + 1])
                            else:
                                nc.scalar.activation(
                                    g[:, half * G:(half + 1) * G], t[:, :],
                                    GATE_FNS[gi])
                        sb[gi] = g
                    x1 = gates.tile([128, 2 * G], f16, tag="x1", name=f"x1{sfx}")
                    x2 = gates.tile([128, 2 * G], f16, tag="x2", name=f"x2{sfx}")
                    x3 = gates.tile([128, 2 * G], f16, tag="x3", name=f"x3{sfx}")
                    s1 = gates.tile([128, 2 * G], f16, tag="s1", name=f"s1{sfx}")
                    tht = gates.tile([128, 2 * G], f16, tag="th",
                                     name=f"th{sfx}")
                    lc = cp3[:, :, g0:g0 + G]
                    rc = cp3[:, :, B + g0:B + g0 + G]
                    nc.vector.tensor_mul(v2(x1), v2(sb[1]), v2(sb[0]))
                    nc.vector.tensor_mul(v2(x2), v2(sb[2]), lc)
                    nc.vector.tensor_mul(v2(x3), v2(sb[3]), rc)
                    nc.vector.tensor_add(v2(s1), v2(x1), v2(x2))
                    cs = v2(c_n)[:, :, g0:g0 + G]
                    nc.vector.tensor_add(cs, v2(s1), v2(x3))
                    nc.scalar.activation(v2(tht), cs, AF.Tanh)
                    nc.vector.tensor_mul(v2(h_n)[:, :, g0:g0 + G],
                                         v2(sb[4]), v2(tht))
                return h_n, c_n

            h, c = h0, c0
            B = LPC
            lvl = 0
            while B > B_STOP:
                B //= 2
                with nc.named_scope(f"L{lvl}_B{B}"):
                    h, c = fm_level(h, c, B, lvl)
                lvl += 1

            nc.sync.dma_start(out[0:128, :], c[:, :])
            nc.scalar.dma_start(out[128:256, :], h[:, :])

    nc.compile()
    return nc


def _get_nc():
    if "nc" not in _CACHE:
        _CACHE["nc"] = _build()
    return _CACHE["nc"]


def kernel(embs, Wx, bx, Wl, Wr, emb_table, _trace=False, _trace_kwargs=None):
    from concourse.bass_utils import run_bass_kernel_spmd

    embs = np.asarray(embs, dtype=np.float32)
    Wx = np.asarray(Wx, dtype=np.float32)
    bx = np.asarray(bx, dtype=np.float32)
    Wl = np.asarray(Wl, dtype=np.float32)
    Wr = np.asarray(Wr, dtype=np.float32)
    emb_table = np.asarray(emb_table, dtype=np.float32)

    WxT = np.ascontiguousarray(Wx.T)                      # [300, 1024]
    WlT = np.ascontiguousarray(Wl.T)                      # [256, 1280]
    WrT = np.ascontiguousarray(Wr.T)

    # Wx chunks with bx folded in as an extra contraction row (row 44 of
    # chunk 2, matching the ones-row in the embedding chunk)
    WxT_ch = []
    for k in range(2):
        WxT_ch.append(np.ascontiguousarray(
            WxT[128 * k:128 * (k + 1)].astype(np.float16)))
    w2 = np.zeros((128, 1024), dtype=np.float16)
    w2[0:44] = WxT[256:300].astype(np.float16)
    w2[44] = bx.astype(np.float16)
    WxT_ch.append(w2)

    # weight images [128, 2*1280] (k-chunks side by side), fp16
    WlT_img = np.ascontiguousarray(
        np.concatenate([WlT[0:128], WlT[128:256]], axis=1).astype(np.float16))
    WrT_img = np.ascontiguousarray(
        np.concatenate([WrT[0:128], WrT[128:256]], axis=1).astype(np.float16))

    # pad-node x-projection, expanded to the 5-gate layout
    px = emb_table[-1] @ WxT + bx                          # [1024]
    px5 = np.concatenate([px[s:s + 256] for s in _PX5SRC]) # [1280]
    px5r = np.ascontiguousarray(px5.reshape(1, 1280).astype(np.float16))
    px5fm = np.ascontiguousarray(px5.reshape(10, 128).T)   # [128, 10]

    perm = _bitrev_perm(LPC)
    in_maps = []
    for d in range(N_CORES):
        shard = embs[d * LPC:(d + 1) * LPC][perm].T.astype(np.float16)
        e2 = np.zeros((128, LPC), dtype=np.float16)
        e2[0:44] = shard[256:300]
        e2[44] = 1.0
        in_maps.append({
            "embsT0": np.ascontiguousarray(shard[0:128]),
            "embsT1": np.ascontiguousarray(shard[128:256]),
            "embsT2": e2,
            "WxT0": WxT_ch[0], "WxT1": WxT_ch[1], "WxT2": WxT_ch[2],
            "WlT": WlT_img, "WrT": WrT_img,
            "px5fm": px5fm, "px5r": px5r,
        })

    nc = _get_nc()
    res = run_bass_kernel_spmd(nc, in_maps, list(range(N_CORES)),
                               trace=_trace, **(_trace_kwargs or {}))
    _CACHE["last_result"] = res

    # ---- unshard: un-bit-reverse, then fold the remaining levels ----
    rperm = _bitrev_perm(B_STOP)  # position p holds node rperm[p]
    cs, hs = [], []
    for d in range(N_CORES):
        o = np.asarray(res.results[d]["out"], dtype=np.float32)
        cf = o[0:128].reshape(128, 2, B_STOP)
        hf = o[128:256].reshape(128, 2, B_STOP)
        c_nm = np.concatenate([cf[:, 0, :], cf[:, 1, :]], axis=0).T  # [B,256]
        h_nm = np.concatenate([hf[:, 0, :], hf[:, 1, :]], axis=0).T
        inv = np.empty(B_STOP, dtype=np.int64)
        inv[rperm] = np.arange(B_STOP)
        cs.append(c_nm[inv])   # node order
        hs.append(h_nm[inv])
    c = np.concatenate(cs, axis=0)  # [512, 256]
    h = np.concatenate(hs, axis=0)
    m = MEM_DIM

    def sig(x):
        return 1.0 / (1.0 + np.exp(-x))

    while c.shape[0] > 1:
        lg = h[0::2] @ WlT
        rg = h[1::2] @ WrT
        u = np.tanh(px[0:m] + lg[:, 0:m] + rg[:, 0:m])
        i = sig(px[m:2 * m] + lg[:, m:2 * m] + rg[:, m:2 * m])
        lf = sig(px[2 * m:3 * m] + lg[:, 2 * m:3 * m] + rg[:, 2 * m:3 * m])
        rf = sig(px[2 * m:3 * m] + lg[:, 3 * m:4 * m] + rg[:, 3 * m:4 * m])
        o = sig(px[3 * m:4 * m] + lg[:, 4 * m:5 * m] + rg[:, 4 * m:5 * m])
        c = i * u + lf * c[0::2] + rf * c[1::2]
        h = o * np.tanh(c)
    return np.stack([c, h]).astype(np.float32)


# revision 30
# speedup vs baseline: 1.3012x; 1.2885x over previous
"""BinaryTreeLSTM on 8 Trainium2 NeuronCores.

Data-parallel over the leaf batch: core d owns leaves [1024d, 1024d+1024)
in BIT-REVERSED order and folds its subtree feature-major through the leaf
projection plus 3 merge levels (1024 -> 128 nodes); the 8x128 per-core
subtree roots are combined on the host for the remaining (tiny, serial,
latency-bound) top-of-tree levels.

Bit-reversal makes every level's left children land at free columns [0:B]
and right children at [B:2B], so all levels use identical feature-major
compute: state is [128 partitions = m-features, 2 chunks, nodes], weights
are the stationary matmul operand (bf16 -> fast weight load), h streams as
the moving operand (f32r, single-pass PE), and child reads are contiguous
slices. No transposes, no SBUF-to-SBUF gathers, no node-major regime.

Bias handling: bx is folded into the leaf matmul via an augmented ones-row
in the embedding chunk / bx-row in the Wx chunk; the internal-node pad
projection px is host-precomputed and applied via the ACT per-partition
bias (wide levels) or a rank-1 PE pass (narrow levels, prefetchable).
"""

import numpy as np

IN_DIM = 300
MEM_DIM = 256
N_LEAVES = 8192
N_CORES = 8
LPC = N_LEAVES // N_CORES  # 1024 leaves per core
B_STOP = 256               # per-core nodes returned to the host
GL = 256                   # leaf/level node-chunk size

# 5-gate order [u, i, lf, rf, o]; lf and rf share the fx slice of px
_PX5SRC = [0, 256, 512, 512, 768]

_CACHE = {}


def _bitrev_perm(n):
    bits = n.bit_length() - 1
    p = np.arange(n)
    r = np.zeros(n, dtype=np.int64)
    for b in range(bits):
        r |= ((p >> b) & 1) << (bits - 1 - b)
    return r


def _build():
    import concourse.bacc as bacc
    import concourse.mybir as mybir
    import concourse.tile as tile

    f32 = mybir.dt.float32
    f32r = mybir.dt.float32r
    f16 = mybir.dt.float16
    AF = mybir.ActivationFunctionType

    nc = bacc.Bacc("TRN2", target_bir_lowering=False, debug=False,
                   num_devices=N_CORES)

    # k-chunked inputs (separate tensors => DMA/dependency granularity)
    embsT = [nc.dram_tensor(f"embsT{k}", [128, LPC], f16,
                            kind="ExternalInput").ap() for k in range(3)]
    WxT = [nc.dram_tensor(f"WxT{k}", [128, 1024], f16,
                          kind="ExternalInput").ap() for k in range(3)]
    WlT = nc.dram_tensor("WlT", [128, 2 * 1280], f16, kind="ExternalInput").ap()
    WrT = nc.dram_tensor("WrT", [128, 2 * 1280], f16, kind="ExternalInput").ap()
    px5fm = nc.dram_tensor("px5fm", [128, 10], f32, kind="ExternalInput").ap()
    px5r = nc.dram_tensor("px5r", [1, 1280], f16, kind="ExternalInput").ap()
    out = nc.dram_tensor("out", [256, 2 * B_STOP], f32, kind="ExternalOutput").ap()

    with tile.TileContext(nc) as tc:
        with (
            tc.tile_pool(name="const", bufs=1) as const,
            tc.tile_pool(name="state", bufs=1) as state,
            tc.tile_pool(name="gates", bufs=2) as gates,
            tc.tile_pool(name="psum", bufs=1, space="PSUM") as psum,
        ):
            v2 = lambda t: t.rearrange("p (c n) -> p c n", c=2)

            # HAM warm-up source: memset, no DMA dependency, scheduled at
            # the very front so dummy matmuls can warm the PE clock gate
            # while input DMAs stream
            warm_sb = const.tile([128, 1024], f16, tag="warm")
            warm_ps = psum.tile([128, 512], f32, tag="u", bufs=2, name="warm")
            with tc.high_priority():
                nc.vector.memset(warm_sb[:, :], 1.0)
                for wi in range(7):
                    nc.tensor.matmul(warm_ps[:, :], warm_sb[:, 0:128],
                                     warm_sb[:, 0:512],
                                     start=(wi == 0), stop=(wi == 6))

            # ---- input DMAs: leaf tensors first, spread across queues ----
            WxT_sb = [const.tile([128, 1024], f16, name=f"wx{k}",
                             tag=f"wx{k}") for k in range(3)]
            embsT_sb = [const.tile([128, LPC], f16, name=f"em{k}",
                        tag=f"em{k}") for k in range(3)]
            # first-needed-first per queue; embsT ships as 512-col halves
            # so the first leaf chunk's inputs land early
            nc.scalar.dma_start(WxT_sb[0][:, :], WxT[0][:, :])
            nc.sync.dma_start(WxT_sb[1][:, :], WxT[1][:, :])
            for k in range(3):
                nc.gpsimd.dma_start(embsT_sb[k][:, 0:512], embsT[k][:, 0:512])
            nc.scalar.dma_start(WxT_sb[2][:, :], WxT[2][:, :])
            for k in range(3):
                nc.sync.dma_start(embsT_sb[k][:, 512:1024],
                                  embsT[k][:, 512:1024])
            WlT_sb = const.tile([128, 2 * 1280], f16, tag="wl")
            WrT_sb = const.tile([128, 2 * 1280], f16, tag="wr")
            px5fm_sb = const.tile([128, 10], f32, tag="pxf")
            px5r_sb = const.tile([1, 1280], f16, tag="pxr")
            nc.scalar.dma_start(WlT_sb[:, :], WlT[:, :])
            nc.gpsimd.dma_start(WrT_sb[:, :], WrT[:, :])
            nc.sync.dma_start(px5fm_sb[:, :], px5fm[:, :])
            nc.sync.dma_start(px5r_sb[:, :], px5r[:, :])

            ones_sb = warm_sb  # all-ones f16, used by the rank-1 px pass
            GATE_FNS = [AF.Tanh, AF.Sigmoid, AF.Sigmoid, AF.Sigmoid, AF.Sigmoid]
            GTAG = ["u", "i", "lf", "rf", "o"]

            # ---- leaf phase: 1024 leaves -> c0, h0 ----
            # ki-outer so matmuls start as soon as chunk-0 DMAs land; each
            # gate's PSUM tile holds both halves (one bank) in a single
            # accumulation group (start only on the very first matmul)
            c0 = state.tile([128, 2 * LPC], f16, name="c_leaf", tag="c_leaf")
            h0 = state.tile([128, 2 * LPC], f16, name="h_leaf", tag="h_leaf")
            c0_3, h0_3 = v2(c0), v2(h0)
            KR = [128, 128, 45]  # rows per k-chunk (chunk 2: 44 data + bias)
            GLF = 512
            LEAF_G = (("u", 0, AF.Tanh), ("i", 1, AF.Sigmoid),
                      ("o", 3, AF.Sigmoid))
            with nc.named_scope("leaf"):
                for sg in range(LPC // GLF):
                    ps = {}
                    for gname, gm, fn in LEAF_G:
                        for half in range(2):
                            ps[gname, half] = psum.tile(
                                [128, GLF], f32, tag=gname,
                                name=f"ps_{gname}{sg}_{half}", bufs=2)
                    for ki in range(3):
                        for gname, gm, fn in LEAF_G:
                            for half in range(2):
                                m = gm * 2 + half
                                nc.tensor.matmul(
                                    ps[gname, half][:, :],
                                    WxT_sb[ki][0:KR[ki], m * 128:(m + 1) * 128],
                                    embsT_sb[ki][0:KR[ki],
                                                 sg * GLF:(sg + 1) * GLF],
                                    start=(ki == 0), stop=(ki == 2))
                    sb = {}
                    for gname, gm, fn in LEAF_G:
                        t = gates.tile([128, 2 * GLF], f16, tag=gname,
                                       name=f"g_{gname}{sg}")
                        for half in range(2):
                            nc.scalar.activation(
                                t[:, half * GLF:(half + 1) * GLF],
                                ps[gname, half][:, :], fn)
                        sb[gname] = t
                    tht = gates.tile([128, 2 * GLF], f16, tag="th", name=f"th{sg}")
                    cs = c0_3[:, :, sg * GLF:(sg + 1) * GLF]
                    nc.vector.tensor_mul(cs, v2(sb["i"]), v2(sb["u"]))
                    nc.scalar.activation(v2(tht), cs, AF.Tanh)
                    nc.vector.tensor_mul(h0_3[:, :, sg * GLF:(sg + 1) * GLF],
                                         v2(sb["o"]), v2(tht))

            # ---- merge levels, all feature-major ----
            def fm_level(h_prev, c_prev, B, lvl):
                last = (B == B_STOP)
                h_n = state.tile([128, 2 * B], f32 if last else f16,
                                 name=f"h{lvl}", tag=f"h{lvl}")
                c_n = state.tile([128, 2 * B], f32 if last else f16,
                                 name=f"c{lvl}", tag=f"c{lvl}")
                hp3, cp3 = v2(h_prev), v2(c_prev)
                use_bias = B >= 256
                CH = 256 if B > 256 else (128 if B > 128 else B)
                for g0 in range(0, B, CH):
                    G = min(CH, B - g0)
                    sfx = f"{lvl}_{g0}"
                    sb = {}
                    for gi in range(5):
                        g = gates.tile([128, 2 * G], f16, tag=GTAG[gi],
                                       name=f"g_{GTAG[gi]}{sfx}")
                        for half in range(2):
                            m = gi * 2 + half
                            t = psum.tile([128, G], f32, tag=GTAG[gi],
                                          name=f"ps{GTAG[gi]}{sfx}_{half}",
                                          bufs=2 if gi in (0, 1, 4) else 1)
                            if not use_bias:
                                nc.tensor.matmul(
                                    t[:, :],
                                    px5r_sb[0:1, m * 128:(m + 1) * 128],
                                    ones_sb[0:1, 0:G],
                                    start=True, stop=False)
                            for ki in range(4):
                                side, kc = ki // 2, ki % 2
                                W = WlT_sb if side == 0 else WrT_sb
                                nc.tensor.matmul(
                                    t[:, :],
                                    W[:, kc * 1280 + m * 128:
                                      kc * 1280 + (m + 1) * 128],
                                    hp3[:, kc, side * B + g0:side * B + g0 + G],
                                    start=(use_bias and ki == 0),
                                    stop=(ki == 3))
                            if use_bias:
                                nc.scalar.activation(
                                    g[:, half * G:(half + 1) * G], t[:, :],
                                    GATE_FNS[gi],
                                    bias=px5fm_sb[:, gi * 2 + half:
                                                  gi * 2 + half + 1])
                            else:
                                nc.scalar.activation(
                                    g[:, half * G:(half + 1) * G], t[:, :],
                                    GATE_FNS[gi])
                        sb[gi] = g
                    x1 = gates.tile([128, 2 * G], f16, tag="x1", name=f"x1{sfx}")
                    x2 = gates.tile([128, 2 * G], f16, tag="x2", name=f"x2{sfx}")
                    x3 = gates.tile([128, 2 * G], f16, tag="x3", name=f"x3{sfx}")
                    s1 = gates.tile([128, 2 * G], f16, tag="s1", name=f"s1{sfx}")
                    tht = gates.tile([128, 2 * G], f16, tag="th",
                                     name=f"th{sfx}")
                    lc = cp3[:, :, g0:g0 + G]
                    rc = cp3[:, :, B + g0:B + g0 + G]
                    nc.vector.tensor_mul(v2(x1), v2(sb[1]), v2(sb[0]))
                    nc.vector.tensor_mul(v2(x2), v2(sb[2]), lc)
                    nc.vector.tensor_mul(v2(x3), v2(sb[3]), rc)
                    nc.vector.tensor_add(v2(s1), v2(x1), v2(x2))
                    cs = v2(c_n)[:, :, g0:g0 + G]
                    nc.vector.tensor_add(cs, v2(s1), v2(x3))
                    nc.scalar.activation(v2(tht), cs, AF.Tanh)
                    nc.vector.tensor_mul(v2(h_n)[:, :, g0:g0 + G],
                                         v2(sb[4]), v2(tht))
                return h_n, c_n

            h, c = h0, c0
            B = LPC
            lvl = 0
            while B > B_STOP:
                B //= 2
                with nc.named_scope(f"L{lvl}_B{B}"):
                    h, c = fm_level(h, c, B, lvl)
                lvl += 1

            nc.sync.dma_start(out[0:128, :], c[:, :])
            nc.scalar.dma_start(out[128:256, :], h[:, :])

    nc.compile()
    return nc


def _get_nc():
    if "nc" not in _CACHE:
        _CACHE["nc"] = _build()
    return _CACHE["nc"]


def kernel(embs, Wx, bx, Wl, Wr, emb_table, _trace=False, _trace_kwargs=None):
    from concourse.bass_utils import run_bass_kernel_spmd

    embs = np.asarray(embs, dtype=np.float32)
    Wx = np.asarray(Wx, dtype=np.float32)
    bx = np.asarray(bx, dtype=np.float32)
    Wl = np.asarray(Wl, dtype=np.float32)
    Wr = np.asarray(Wr, dtype=np.float32)
    emb_table = np.asarray(emb_table, dtype=np.float32)

    WxT = np.ascontiguousarray(Wx.T)                      # [300, 1024]
    WlT = np.ascontiguousarray(Wl.T)                      # [256, 1280]
    WrT = np.ascontiguousarray(Wr.T)

    # Wx chunks with bx folded in as an extra contraction row (row 44 of
    # chunk 2, matching the ones-row in the embedding chunk)
    WxT_ch = []
    for k in range(2):
        WxT_ch.append(np.ascontiguousarray(
            WxT[128 * k:128 * (k + 1)].astype(np.float16)))
    w2 = np.zeros((128, 1024), dtype=np.float16)
    w2[0:44] = WxT[256:300].astype(np.float16)
    w2[44] = bx.astype(np.float16)
    WxT_ch.append(w2)

    # weight images [128, 2*1280] (k-chunks side by side), fp16
    WlT_img = np.ascontiguousarray(
        np.concatenate([WlT[0:128], WlT[128:256]], axis=1).astype(np.float16))
    WrT_img = np.ascontiguousarray(
        np.concatenate([WrT[0:128], WrT[128:256]], axis=1).astype(np.float16))

    # pad-node x-projection, expanded to the 5-gate layout
    px = emb_table[-1] @ WxT + bx                          # [1024]
    px5 = np.concatenate([px[s:s + 256] for s in _PX5SRC]) # [1280]
    px5r = np.ascontiguousarray(px5.reshape(1, 1280).astype(np.float16))
    px5fm = np.ascontiguousarray(px5.reshape(10, 128).T)   # [128, 10]

    perm = _bitrev_perm(LPC)
    in_maps = []
    for d in range(N_CORES):
        shard = embs[d * LPC:(d + 1) * LPC][perm].T.astype(np.float16)
        e2 = np.zeros((128, LPC), dtype=np.float16)
        e2[0:44] = shard[256:300]
        e2[44] = 1.0
        in_maps.append({
            "embsT0": np.ascontiguousarray(shard[0:128]),
            "embsT1": np.ascontiguousarray(shard[128:256]),
            "embsT2": e2,
            "WxT0": WxT_ch[0], "WxT1": WxT_ch[1], "WxT2": WxT_ch[2],
            "WlT": WlT_img, "WrT": WrT_img,
            "px5fm": px5fm, "px5r": px5r,
        })

    nc = _get_nc()
    res = run_bass_kernel_spmd(nc, in_maps, list(range(N_CORES)),
                               trace=_trace, **(_trace_kwargs or {}))
    _CACHE["last_result"] = res

    # ---- unshard: un-bit-reverse, then fold the remaining levels ----
    rperm = _bitrev_perm(B_STOP)  # position p holds node rperm[p]
    cs, hs = [], []
    for d in range(N_CORES):
        o = np.asarray(res.results[d]["out"], dtype=np.float32)
        cf = o[0:128].reshape(128, 2, B_STOP)
        hf = o[128:256].reshape(128, 2, B_STOP)
        c_nm = np.concatenate([cf[:, 0, :], cf[:, 1, :]], axis=0).T  # [B,256]
        h_nm = np.concatenate([hf[:, 0, :], hf[:, 1, :]], axis=0).T
        inv = np.empty(B_STOP, dtype=np.int64)
        inv[rperm] = np.arange(B_STOP)
        cs.append(c_nm[inv])   # node order
        hs.append(h_nm[inv])
    c = np.concatenate(cs, axis=0)  # [512, 256]
    h = np.concatenate(hs, axis=0)
    m = MEM_DIM

    def sig(x):
        return 1.0 / (1.0 + np.exp(-x))

    while c.shape[0] > 1:
        lg = h[0::2] @ WlT
        rg = h[1::2] @ WrT
        u = np.tanh(px[0:m] + lg[:, 0:m] + rg[:, 0:m])
        i = sig(px[m:2 * m] + lg[:, m:2 * m] + rg[:, m:2 * m])
        lf = sig(px[2 * m:3 * m] + lg[:, 2 * m:3 * m] + rg[:, 2 * m:3 * m])
        rf = sig(px[2 * m:3 * m] + lg[:, 3 * m:4 * m] + rg[:, 3 * m:4 * m])
        o = sig(px[3 * m:4 * m] + lg[:, 4 * m:5 * m] + rg[:, 4 * m:5 * m])
        c = i * u + lf * c[0::2] + rf * c[1::2]
        h = o * np.tanh(c)
    return np.stack([c, h]).astype(np.float32)
